# revision 1
# baseline (speedup 1.0000x reference)
"""Trainium2 Bass kernel for nn_MultiHeadAttention_65910568125151 (v5).

B=4, S=1024, D=1024, H=16 heads (dk=64). 8 NeuronCores, sharded
batch x head-half: core c handles batch c//2 and heads (c%2)*8..+8.

All matmuls fp16 (measured on hw: fp8 DoubleRow streams at the same
~1 cycle/output-row as fp16, so fp8 only helps when it halves the
pass count, which the precision budget never allows here).

Per-core program (PSUM always fp32):
  phase 1: qT/kT [dk-major, S] and v [tok-major, dk+ones] projections
  phase 2 per head h, per key-tile kt:
    scoresT psum = kT_h.T @ qT_h          (K=64)
    e16 = Exp(0.125*scores - 2)           (ACT, whole [128,1024] tile,
                                           bias -2 cancels in softmax)
    e16 *= cfac16[kt]                     (DVE mult; cfac is host-side
        exp(-lam*probT) * (maskT != 0): exact zeros ARE the mask --
        no fp32 comb-add, no -1e9 logits)
    pa += [v_h | ones] @ e16              (fused attn + rowsum)
  normalize per head: rowsum row -> PE-transpose [2,128] chunks into
    [128,16] (single accumulation group in a ps-pool psum buffer),
    DVE reciprocal along the free dim (~0.25us vs 6.5us lane-serial),
    PE-transpose back, DRAM partition-broadcast, DVE multiply.
  phase 3: out16 = attnT.T @ (Wo.T)loc, psum drained on DVE.
Host: out[b] = partial[2b] + partial[2b+1] + (bo + Wo@bv); bk drops
(per-q scores shift, softmax invariant).
"""

import numpy as np

_B, _S, _D = 4, 1024, 1024
_P = 128
_DL = 512          # local hidden (8 heads x 64)
_HL = 8            # local heads
_DK = 64
_KC = _D // _P     # 8 contraction chunks of 128
_MQ = _DL // _P    # 4 m-tiles for qT/kT
_MT = _S // _P     # 8 token tiles
_KO = _DL // _P    # 4 contraction chunks, out-proj
_NH = (0, 512)     # free-dim halves


def _build_program_v5():
    from collections import deque

    import concourse.mybir as mybir
    import concourse.tile as tile
    from concourse import bacc
    from concourse.alu_op_type import AluOpType

    f32 = mybir.dt.float32
    f16 = mybir.dt.float16
    Copy = mybir.ActivationFunctionType.Copy
    Exp = mybir.ActivationFunctionType.Exp
    S, DL, P, HL, MT, MQ, KC, KO = _S, _DL, _P, _HL, _MT, _MQ, _KC, _KO

    nc = bacc.Bacc()

    xq_d = nc.dram_tensor("xq16", [_D, S], f16, kind="ExternalInput")
    xk_d = nc.dram_tensor("xk16", [_D, S], f16, kind="ExternalInput")
    xv_d = nc.dram_tensor("xv16", [_D, S], f16, kind="ExternalInput")
    wq_d = nc.dram_tensor("wq16", [_D, DL], f16, kind="ExternalInput")
    wk_d = nc.dram_tensor("wk16", [_D, DL], f16, kind="ExternalInput")
    wv_d = nc.dram_tensor("wv16", [_D, DL], f16, kind="ExternalInput")
    wo_d = nc.dram_tensor("wo16", [DL, _D], f16, kind="ExternalInput")
    cf_d = nc.dram_tensor("cf16", [S, S], f16, kind="ExternalInput")
    id_d = nc.dram_tensor("ident", [P, P], f32, kind="ExternalInput")
    out_d = nc.dram_tensor("out16", [S, _D], f16, kind="ExternalOutput")

    with tile.TileContext(nc) as tc:
        with (
            tc.tile_pool(name="px", bufs=2) as px,
            tc.tile_pool(name="pqk", bufs=1) as pqk,
            tc.tile_pool(name="psm", bufs=4) as psm,
            tc.tile_pool(name="prs", bufs=2) as prs,
            tc.tile_pool(name="pdr", bufs=3, space="DRAM") as pdr,
        ):
            # ---- persistent sbuf tiles ----
            # stacked scores operands [128, head, block(2), S]:
            #   q (rhs):  block0 = [q8; qr8], block1 = [q8; qr8]
            #   k (lhsT): block0 = [k8; k8],  block1 = [kr8; 0]
            f8 = mybir.dt.float8e4
            DR = mybir.MatmulPerfMode.DoubleRow
            qs_t = pqk.tile([P, HL, 2, S], f8, tag="qs")
            ks_t = pqk.tile([P, HL, 2, S], f8, tag="ks")
            v_t = pqk.tile([P, MT, HL, _DK + 1], f16, tag="v")
            attnT_t = pqk.tile([P, KO, S], f16, tag="attnT")
            cf_t = pqk.tile([P, MT, S], f16, tag="cf")
            id_t = pqk.tile([P, P], f32, tag="ident")
            ebias_t = pqk.tile([P, 1], f32, tag="ebias")
            z64_t = pqk.tile([64, P], f32, tag="z64")
            z_t = pqk.tile([P, S], f32, tag="z")

            def load(d, n_chunks, ncols, nm, pool=None):
                t = (pool or pqk).tile([P, n_chunks, ncols], f16, tag=nm)
                nc.sync.dma_start(
                    t[:], d[:].rearrange("(c p) n -> p c n", p=P))
                return t

            # q-proj inputs first so the first matmul starts ~10us in;
            # everything else follows on the rings
            wq_t = load(wq_d, KC, DL, "wq")
            xq_t = load(xq_d, KC, S, "x", pool=px)
            wk_t = load(wk_d, KC, DL, "wk")
            xk_t = load(xk_d, KC, S, "x", pool=px)
            wv_t = load(wv_d, KC, DL, "wv")
            xv_t = load(xv_d, KC, S, "x", pool=px)
            wo_t = load(wo_d, KO, _D, "wo")
            nc.sync.dma_start(id_t[:], id_d[:])
            nc.sync.dma_start(
                cf_t[:], cf_d[:].rearrange("(c p) n -> p c n", p=P))

            # init on gpsimd: off the DVE, overlaps the input DMA
            nc.gpsimd.memset(v_t[:, :, :, _DK:_DK + 1], 1.0)
            nc.gpsimd.memset(ks_t[64:128, :, 1, :], 0.0)
            nc.gpsimd.memset(ebias_t[:], -2.0)
            nc.gpsimd.memset(z64_t[:], 0.0)
            nc.gpsimd.memset(z_t[:], 0.0)

            # ---- phase 1: projections (fp16) ----
            with tc.tile_pool(name="pp1", bufs=2, space="PSUM") as pp1:
                for which, w_t, x_t, dst in (
                    ("q", wq_t, xq_t, qs_t), ("k", wk_t, xk_t, ks_t),
                ):
                    for m in range(MQ):
                        pq = pp1.tile([P, S], f32, tag="pq")
                        for kc in range(KC):
                            lhsT = w_t[:, kc, m * P:(m + 1) * P]
                            for o in _NH:
                                nc.tensor.matmul(
                                    pq[:, o:o + 512], lhsT,
                                    x_t[:, kc, o:o + 512],
                                    start=(kc == 0), stop=(kc == KC - 1),
                                )
                        # fp8 value + raw-scale fp8 residual, relaid per
                        # m-tile (heads 2m, 2m+1) through DRAM into the
                        # stacked DoubleRow layout
                        b8 = psm.tile([P, S], f8, tag="b8", bufs=2)
                        r8 = psm.tile([P, S], f8, tag="r8", bufs=2)
                        nc.scalar.activation(b8[:], pq[:], Copy)
                        nc.vector.scalar_tensor_tensor(
                            out=r8[:], in0=pq[:], scalar=1.0, in1=b8[:],
                            op0=AluOpType.mult, op1=AluOpType.subtract,
                        )
                        bd = pdr.tile([P, S], f8, tag="bd")
                        rd = pdr.tile([P, S], f8, tag="rd")
                        nc.sync.dma_start(bd[:], b8[:])
                        nc.sync.dma_start(rd[:], r8[:])
                        bv_ = bd[:].rearrange("(h p) n -> p h n", p=_DK)
                        rv_ = rd[:].rearrange("(h p) n -> p h n", p=_DK)
                        hsl = slice(2 * m, 2 * m + 2)
                        if which == "q":
                            for j in (0, 1):
                                nc.sync.dma_start(dst[0:64, hsl, j, :], bv_)
                                nc.sync.dma_start(dst[64:128, hsl, j, :], rv_)
                        else:
                            nc.sync.dma_start(dst[0:64, hsl, 0, :], bv_)
                            nc.sync.dma_start(dst[64:128, hsl, 0, :], bv_)
                            nc.sync.dma_start(dst[0:64, hsl, 1, :], rv_)

                for mt in range(MT):
                    pv = pp1.tile([P, DL], f32, tag="pv")
                    for kc in range(KC):
                        nc.tensor.matmul(
                            pv[:],
                            xv_t[:, kc, mt * P:(mt + 1) * P],
                            wv_t[:, kc, :],
                            start=(kc == 0), stop=(kc == KC - 1),
                        )
                    nc.scalar.activation(
                        v_t[:, mt, :, 0:_DK],
                        pv[:].rearrange("p (h d) -> p h d", h=HL),
                        Copy,
                    )

            # ---- phase 2: attention per head ----
            # PSUM (7 of 8 banks): ps halves 2x1 + pa 2x2 + rr 1
            rs2_t = pqk.tile([2, S], f32, tag="rs2")
            nc.gpsimd.memset(rs2_t[:], 1.0)
            with tc.tile_pool(name="pp2", bufs=2, space="PSUM") as pp2:
                pending = deque()

                def norm_tail(h, pa):
                    h2, ko = h % 2, h // 2
                    # rowsum -> rs2 row 0 (row 1 stays 1.0 filler so the
                    # [2,128] transposes and reciprocal see finite junk)
                    nc.vector.scalar_tensor_tensor(
                        out=rs2_t[0:1, :], in0=pa[64:65, :],
                        scalar=1.0, in1=z_t[0:1, :],
                        op0=AluOpType.mult, op1=AluOpType.add)
                    # transpose rowsum chunks into one psum bank (single
                    # accumulation group), cheap reciprocal along the free
                    # dim, transpose back, DRAM broadcast, multiply
                    rr = pp2.tile([P, P], f32, tag="rr", bufs=1)
                    for t in range(MT):
                        nc.tensor.matmul(
                            rr[:, 2 * t:2 * t + 2],
                            rs2_t[:, t * P:(t + 1) * P], id_t[0:2, 0:2],
                            is_transpose=True,
                            start=(t == 0), stop=(t == MT - 1),
                            skip_group_check=True,
                        )
                    rcT = prs.tile([P, 16], f32, tag="rcT")
                    nc.vector.reciprocal(rcT[:], rr[:, 0:16])
                    nc.tensor.matmul(
                        rr[0:16, 0:128], rcT[:], id_t[:],
                        is_transpose=True, start=True, stop=True,
                        skip_group_check=True,
                    )
                    rcn = prs.tile([16, P], f32, tag="rcnsb")
                    nc.vector.scalar_tensor_tensor(
                        out=rcn[:], in0=rr[0:16, 0:128], scalar=1.0,
                        in1=z64_t[0:16, :],
                        op0=AluOpType.mult, op1=AluOpType.add)
                    rc_d = pdr.tile([16, P], f32, tag="rcd")
                    nc.sync.dma_start(rc_d[:], rcn[:])
                    rc_b = prs.tile([64, S], f32, tag="rcb")
                    nc.sync.dma_start(
                        rc_b[:].rearrange("p (t q) -> p t q", t=MT),
                        rc_d[:].rearrange("(t h) q -> h t q", h=2)
                            [0:1, :, :].partition_broadcast(64))
                    nc.vector.tensor_tensor(
                        out=attnT_t[h2 * 64:h2 * 64 + 64, ko, :],
                        in0=pa[0:64, :], in1=rc_b[:],
                        op=AluOpType.mult,
                    )

                def flush_one():
                    h, kt, e16, pa = pending.popleft()
                    vh = v_t[:, kt, h, :]
                    for o in _NH:
                        nc.tensor.matmul(
                            pa[:, o:o + 512], vh, e16[:, o:o + 512],
                            start=(kt == 0), stop=(kt == MT - 1),
                        )
                    if kt == MT - 1:
                        norm_tail(h, pa)

                for h in range(HL):
                    kTh = ks_t[:, h, :, :]
                    qTh = qs_t[:, h, :, :]
                    pa = pp2.tile([65, S], f32, tag="pa")
                    for kt in range(MT):
                        eh = psm.tile([P, S], f16, tag="eh", bufs=3)
                        lhsT = kTh[:, :, kt * P:(kt + 1) * P]
                        for o in _NH:
                            ps_ = pp2.tile([P, 512], f32, tag="ps",
                                           bufs=3)
                            nc.tensor.matmul(
                                ps_[:], lhsT, qTh[:, :, o:o + 512],
                                start=True, stop=True, perf_mode=DR,
                            )
                            nc.scalar.activation(
                                eh[:, o:o + 512], ps_[:], Exp,
                                scale=0.125, bias=ebias_t[:])
                        e16 = psm.tile([P, S], f16, tag="e16", bufs=4)
                        nc.vector.tensor_tensor(
                            out=e16[:], in0=eh[:], in1=cf_t[:, kt, :],
                            op=AluOpType.mult,
                        )
                        pending.append((h, kt, e16, pa))
                        if len(pending) > 2:
                            flush_one()
                while pending:
                    flush_one()

            # ---- phase 3: output projection (fp16) ----
            with tc.tile_pool(name="pp3", bufs=2, space="PSUM") as pp3:
                for mt in range(MT):
                    po = pp3.tile([P, _D], f32, tag="po")
                    for ko in range(KO):
                        lhsT = attnT_t[:, ko, mt * P:(mt + 1) * P]
                        for o in _NH:
                            nc.tensor.matmul(
                                po[:, o:o + 512], lhsT, wo_t[:, ko, o:o + 512],
                                start=(ko == 0), stop=(ko == KO - 1),
                            )
                    o_sb = psm.tile([P, _D], f16, tag="osb", bufs=2)
                    nc.scalar.activation(o_sb[:], po[:], Copy)
                    nc.sync.dma_start(out_d[mt * P:(mt + 1) * P, :], o_sb[:])

    nc.compile()
    return nc


_PROG_CACHE = {}


def _get_program_v5():
    if "v5" not in _PROG_CACHE:
        _PROG_CACHE["v5"] = _build_program_v5()
    return _PROG_CACHE["v5"]


def _prepare_in_maps_v5(Qx, Kx, Vx, prob_phn, mask, lambda_val,
                        Wq, bq, Wk, bk, Wv, bv, Wo, bo):
    f32 = np.float32
    f16 = np.float16
    Qx = np.asarray(Qx, f32)
    Kx = np.asarray(Kx, f32)
    Vx = np.asarray(Vx, f32)
    prob = np.asarray(prob_phn, f32)
    mask_np = np.asarray(mask)
    lam = float(np.asarray(lambda_val))

    QxT = np.ascontiguousarray(Qx.transpose(0, 2, 1)).astype(f16)
    KxT = np.ascontiguousarray(Kx.transpose(0, 2, 1)).astype(f16)
    VxT = np.ascontiguousarray(Vx.transpose(0, 2, 1)).astype(f16)
    WqT = np.ascontiguousarray(np.asarray(Wq, f32).T).astype(f16)
    WkT = np.ascontiguousarray(np.asarray(Wk, f32).T).astype(f16)
    WvT = np.ascontiguousarray(np.asarray(Wv, f32).T).astype(f16)
    WoT = np.ascontiguousarray(np.asarray(Wo, f32).T).astype(f16)

    cf = np.exp(-lam * prob) if lam > 0 else np.ones_like(prob)
    cf = cf * (mask_np.transpose(0, 2, 1) != 0)
    cf16 = cf.astype(f16)

    in_maps = []
    for c in range(8):
        b, hh = divmod(c, 2)
        sl = slice(hh * _DL, (hh + 1) * _DL)
        m = {
            "xq16": QxT[b], "xk16": KxT[b], "xv16": VxT[b], "cf16": cf16[b],
            "wq16": np.ascontiguousarray(WqT[:, sl]),
            "wk16": np.ascontiguousarray(WkT[:, sl]),
            "wv16": np.ascontiguousarray(WvT[:, sl]),
            "wo16": np.ascontiguousarray(WoT[sl, :]),
            "ident": np.eye(128, dtype=f32),
        }
        in_maps.append(m)
    bo_eff = np.asarray(bo, f32) + np.asarray(Wo, f32) @ np.asarray(bv, f32)
    return in_maps, mask_np, bo_eff


def _run_v5(trace=False, tmpdir=None, **inputs):
    from concourse.bass_utils import run_bass_kernel_spmd

    in_maps, mask_np, bo_eff = _prepare_in_maps_v5(**inputs)
    nc = _get_program_v5()
    br = run_bass_kernel_spmd(nc, in_maps, list(range(8)), trace=trace,
                              tmpdir=tmpdir)
    out = np.empty((_B, _S, _D), np.float32)
    for b in range(_B):
        out[b] = (br.results[2 * b]["out16"].astype(np.float32)
                  + br.results[2 * b + 1]["out16"].astype(np.float32))
    out += bo_eff
    return (out, mask_np), br


def kernel(**inputs):
    (out, mask_np), _ = _run_v5(trace=False, **inputs)
    return out, mask_np


_run = _run_v5
_prepare_in_maps_v3 = _prepare_in_maps_v5
_get_program_v3 = _get_program_v5

